# revision 27
# baseline (speedup 1.0000x reference)
"""AtariGRU Trainium2 kernel: B=256,T=128,D=U=512, Keras GRU (reset_after) with
done/step-counter resets. Data-parallel over batch on 8 cores; time recurrence
parallelized over T-chunks (len 16 + halo 16) exploiting the guaranteed reset
every <=16 steps.

Fully transposed-state formulation: the hidden state lives as h^T
[feature, row] so the recurrent matmul (stationary = U chunks, moving =
h^T) needs no per-step transposes; gate math runs in the same layout and
emits the next h^T directly. Halo reads become free-dim offsets. bf16
matmuls, f32 PSUM.

kernel(**inputs) takes FULL inputs, returns (sequences, state_f, step_f).
"""
import sys
import numpy as np

for _p in ("/opt/trn_rl_repo", "/root/.axon_site/_ro/trn_rl_repo"):
    if _p not in sys.path:
        sys.path.insert(0, _p)

import ml_dtypes

BF16 = ml_dtypes.bfloat16

# problem constants (hardcoded per harness rules)
B, T, D, UN = 256, 128, 512, 512
NCORES = 8
BC = B // NCORES          # 32 batch rows per core
CH = 16                   # chunk length
HALO = 16                 # halo length (max reset gap)
S = CH + HALO             # steps per chunk
NCHUNK = T // CH          # chunks
NGRP = NCHUNK // 4        # chunk-groups of 4 chunks x 32 rows = 128 rows
CPG = NCHUNK // NGRP      # 4 chunks per group
G3 = 3 * UN               # 1536
MEMORY_SIZE = 16

_nc_cache = {}


def _build_nc():
    import concourse.bacc as bacc
    import concourse.mybir as mybir
    import concourse.tile as tile
    from contextlib import ExitStack

    dt = mybir.dt
    AF = mybir.ActivationFunctionType
    f32 = dt.float32
    bf = dt.bfloat16

    nc = bacc.Bacc(None, target_bir_lowering=False)

    # xT[g, tl, p=d%128, (k=d//128, r)] = x[b(r), t(g,cl(r),tl), d]
    xT_d = nc.declare_dram_parameter("xT", [NGRP, CH, 128, 512], bf, isOutput=False)
    W_d = nc.declare_dram_parameter("Wm", [D, G3], bf, isOutput=False)
    U_d = nc.declare_dram_parameter("Um", [UN, G3], bf, isOutput=False)
    bxT_d = nc.declare_dram_parameter("bxT", [128, 12], f32, isOutput=False)
    b1h_d = nc.declare_dram_parameter("b1hbc", [128, 512], bf, isOutput=False)
    mb_d = nc.declare_dram_parameter("mask_bc", [NGRP, S, 128, 512], bf,
                                     isOutput=False)
    h0_d = nc.declare_dram_parameter("h0T", [128, 512], bf, isOutput=False)
    # outT[g, tl, m, p, r] = h_new[row r, u = m*128+p] at t(g, cl(r), tl)
    out_d = nc.declare_dram_parameter("outT", [NGRP, CH, 4, 128, 128], bf,
                                      isOutput=True)

    with ExitStack() as ctx:
        tc = ctx.enter_context(tile.TileContext(nc))
        const = ctx.enter_context(tc.tile_pool(name="const", bufs=1))
        xmp = ctx.enter_context(tc.tile_pool(name="xmp", bufs=1))
        work = ctx.enter_context(tc.tile_pool(name="work", bufs=2))
        psx = ctx.enter_context(tc.tile_pool(name="psx", bufs=2, space="PSUM"))
        psr = ctx.enter_context(tc.tile_pool(name="psr", bufs=1, space="PSUM"))

        # ---- constants ----
        U_sb = []
        W_sb = []
        for k in range(4):
            ut = const.tile([128, G3], bf, tag=f"U{k}")
            nc.sync.dma_start(out=ut[:], in_=U_d[k * 128:(k + 1) * 128, :])
            U_sb.append(ut)
            wt = const.tile([128, G3], bf, tag=f"W{k}")
            nc.sync.dma_start(out=wt[:], in_=W_d[k * 128:(k + 1) * 128, :])
            W_sb.append(wt)
        bxT = const.tile([128, 12], f32, tag="bxT")
        nc.sync.dma_start(out=bxT[:], in_=bxT_d[:, :])
        b1hbc = const.tile([128, 512], bf, tag="b1hbc")
        nc.sync.dma_start(out=b1hbc[:], in_=b1h_d[:, :])
        h0T = const.tile([128, 512], bf, tag="h0T")
        nc.sync.dma_start(out=h0T[:], in_=h0_d[:, :])

        ident = const.tile([128, 128], bf, tag="ident")
        nc.gpsimd.memset(ident[:], 0.0)
        nc.gpsimd.affine_select(
            out=ident[:], in_=ident[:],
            compare_op=mybir.AluOpType.not_equal,
            fill=1.0, base=0, pattern=[[-1, 128]], channel_multiplier=1,
        )

        # ---- phase 1: xmT[g][p, (m, tl, r)] = (X@W + bias_x)^T ----
        xm_sb = []
        for g in range(NGRP):
            xm = xmp.tile([128, 12 * CH * 128], bf, tag=f"xm{g}")
            xm_sb.append(xm)

        def xm_sl(g, m, tl, r0, r1):
            base = (m * CH + tl) * 128
            return xm_sb[g][:, base + r0:base + r1]

        NTLB = CH // 4
        for g in range(NGRP):
            for tlb in range(NTLB):
                xt4 = work.tile([128, 2048], bf, tag="xt4", bufs=3)
                # dram [tl, p, (k r)] -> sbuf free (k, tl, r)
                nc.sync.dma_start(
                    out=xt4[:],
                    in_=xT_d[g, tlb * 4:(tlb + 1) * 4, :, :].rearrange(
                        "tl p (k r) -> p k tl r", k=4),
                )
                for m in range(12):
                    ps = psx.tile([128, 512], f32, tag="psx")
                    for k in range(4):
                        nc.tensor.matmul(
                            ps[:],
                            lhsT=W_sb[k][:, m * 128:(m + 1) * 128],
                            rhs=xt4[:, k * 512:(k + 1) * 512],
                            start=(k == 0), stop=(k == 3),
                        )
                    # evict with per-partition bias (bias_x folded in free)
                    for tt in range(4):
                        tl = tlb * 4 + tt
                        nc.scalar.activation(
                            xm_sl(g, m, tl, 0, 128),
                            ps[:, tt * 128:(tt + 1) * 128],
                            AF.Identity, bias=bxT[:, m:m + 1],
                        )

        # ---- phase 2: recurrence ----
        carry = []
        for g in range(NGRP):
            c = work.tile([128, UN], bf, tag=f"carry{g}")
            nc.vector.memset(c[:], 0.0)
            carry.append(c)
        mbs = {}
        for g in range(NGRP):
            mb = work.tile([128, UN], bf, tag="mb", bufs=4, name=f"mb_0_{g}")
            nc.sync.dma_start(out=mb[:], in_=mb_d[g, 0, :, :])
            mbs[g] = mb

        for j in range(S):
            main = j >= HALO
            if main:
                tl = j - HALO
                sh = 0
            else:
                nb = (HALO - 1 - j) // CH + 1
                tl = j - HALO + nb * CH
                sh = 32 * nb          # halo row shift

            # --- matmuls for all groups ---
            Ps = []
            for g in range(NGRP):
                P = psr.tile([128, G3], f32, tag=f"P{g}")
                gp = g - 1 if g > 0 else g
                # one m-region at a time: PSUM start-group zeroing is
                # bank-granular, so never interleave open groups in a bank
                for m in range(12):
                    for k in range(4):
                        nc.tensor.matmul(
                            P[:, m * 128:(m + 1) * 128],
                            lhsT=U_sb[k][:, m * 128:(m + 1) * 128],
                            rhs=carry[g][:, k * 128:(k + 1) * 128],
                            start=(k == 0), stop=(k == 3 and m < 8),
                        )
                    if m >= 8:
                        # b1h fold for the h part
                        nc.tensor.matmul(
                            P[:, m * 128:(m + 1) * 128],
                            lhsT=ident[:],
                            rhs=b1hbc[:, (m - 8) * 128:(m - 7) * 128],
                            start=False, stop=True,
                        )
                Ps.append(P)

            # --- gate chain, position-interleaved across groups ---
            rr_, hcs, zgs, dds, zds, hns = {}, {}, {}, {}, {}, {}

            def fold_pre(g, dst, psl, mlo):
                # dst = P[:, gate part] + xm (rows shifted by sh in halo)
                gp2 = g - 1 if g > 0 else g
                d3 = dst[:].rearrange("p (m r) -> p m r", m=4)
                p3 = Ps[g][:, psl * 512:(psl + 1) * 512].rearrange(
                    "p (m r) -> p m r", m=4)
                x3 = xm_sb[g][:].rearrange(
                    "p (m tl2 r) -> p m tl2 r", m=12, tl2=CH)
                x3p = xm_sb[gp2][:].rearrange(
                    "p (m tl2 r) -> p m tl2 r", m=12, tl2=CH)
                if sh:
                    nc.vector.tensor_add(
                        d3[:, :, sh:128], p3[:, :, sh:128],
                        x3[:, mlo:mlo + 4, tl, 0:128 - sh])
                    nc.vector.tensor_add(
                        d3[:, :, 0:sh], p3[:, :, 0:sh],
                        x3p[:, mlo:mlo + 4, tl, 128 - sh:128])
                else:
                    nc.vector.tensor_add(d3[:], p3[:], x3[:, mlo:mlo + 4, tl, :])

            for g in range(NGRP):
                rp = work.tile([128, UN], bf, tag="rpre", bufs=4,
                               name=f"rp_{j}_{g}")
                fold_pre(g, rp, 1, 4)
                rr_[g] = rp
            for g in range(NGRP):
                rr = work.tile([128, UN], bf, tag="rg", bufs=4, name=f"rg_{j}_{g}")
                nc.scalar.activation(rr[:], rr_[g][:], AF.Sigmoid)
                rr_[g] = rr
            for g in range(NGRP):
                rh = work.tile([128, UN], bf, tag="rh", bufs=4, name=f"rh_{j}_{g}")
                nc.vector.tensor_mul(rh[:], rr_[g][:], Ps[g][:, 1024:1536])
                rr_[g] = rh
            for g in range(NGRP):
                hcp = work.tile([128, UN], bf, tag="hcpre", bufs=4,
                                name=f"hcp_{j}_{g}")
                gp = g - 1 if g > 0 else g

                def xmh3(gg, r0, r1):
                    return xm_sb[gg][:].rearrange(
                        "p (m tl2 r) -> p m tl2 r", m=12, tl2=CH
                    )[:, 8:12, tl, r0:r1]

                hcp3 = hcp[:].rearrange("p (m r) -> p m r", m=4)
                rh3 = rr_[g][:].rearrange("p (m r) -> p m r", m=4)
                if not main:
                    nc.vector.tensor_add(
                        hcp3[:, :, sh:128], rh3[:, :, sh:128],
                        xmh3(g, 0, 128 - sh),
                    )
                    nc.vector.tensor_add(
                        hcp3[:, :, 0:sh], rh3[:, :, 0:sh],
                        xmh3(gp, 128 - sh, 128),
                    )
                else:
                    nc.vector.tensor_add(hcp3[:], rh3[:], xmh3(g, 0, 128))
                rr_[g] = hcp
            for g in range(NGRP):
                hcs[g] = work.tile([128, UN], bf, tag="hc", bufs=4,
                                   name=f"hc_{j}_{g}")
                nc.scalar.activation(hcs[g][:], rr_[g][:], AF.Tanh)
            for g in range(NGRP):
                zp = work.tile([128, UN], bf, tag="zpre", bufs=4,
                               name=f"zp_{j}_{g}")
                fold_pre(g, zp, 0, 0)
                zgs[g] = zp
            for g in range(NGRP):
                zz = work.tile([128, UN], bf, tag="zg", bufs=4, name=f"zg_{j}_{g}")
                nc.scalar.activation(zz[:], zgs[g][:], AF.Sigmoid)
                zgs[g] = zz
            for g in range(NGRP):
                dds[g] = work.tile([128, UN], bf, tag="dd", bufs=4,
                                   name=f"dd_{j}_{g}")
                nc.vector.tensor_sub(dds[g][:], carry[g][:], hcs[g][:])
            for g in range(NGRP):
                zds[g] = work.tile([128, UN], bf, tag="zd", bufs=4,
                                   name=f"zd_{j}_{g}")
                nc.vector.tensor_mul(zds[g][:], zgs[g][:], dds[g][:])
            for g in range(NGRP):
                hns[g] = work.tile([128, UN], bf, tag="hnew", bufs=5,
                                   name=f"hn_{j}_{g}")
                nc.vector.tensor_add(hns[g][:], hcs[g][:], zds[g][:])
            for g in range(NGRP):
                cn = work.tile([128, UN], bf, tag=f"carry{g}")
                nc.vector.tensor_mul(cn[:], hns[g][:], mbs[g][:])
                if j == HALO - 1 and g == 0:
                    c2 = work.tile([128, UN], bf, tag="cinj")
                    nc.vector.tensor_add(c2[:], cn[:], h0T[:])
                    cn = c2
                carry[g] = cn
            # prefetch next step's mask tiles; emit outputs
            if j < S - 1:
                for g in range(NGRP):
                    mb = work.tile([128, UN], bf, tag="mb", bufs=4,
                                   name=f"mb_{j + 1}_{g}")
                    nc.sync.dma_start(out=mb[:], in_=mb_d[g, j + 1, :, :])
                    mbs[g] = mb
            if main:
                for g in range(NGRP):
                    nc.gpsimd.dma_start(
                        out=out_d[g, tl, :, :, :].rearrange("m p r -> p m r"),
                        in_=hns[g][:],
                    )

    nc.compile()
    return nc


def _host_scan(dones, step0):
    """reset mask r[B,T] (1=reset after step t) and final step counter."""
    Bn, Tn = dones.shape
    s = step0[:, 0].astype(np.int64).copy()
    r = np.zeros((Bn, Tn), dtype=bool)
    for t in range(Tn):
        s_new = s + 1
        reset = (dones[:, t] == 1) | (s_new % MEMORY_SIZE == 0)
        r[:, t] = reset
        s = np.where(reset, 0, s_new)
    return r, s.astype(np.int32)[:, None]


def _make_in_maps(inputs, dones, state, step, W, U, b):
    r, step_f = _host_scan(dones, step)
    mask_f = (~r).astype(np.float32)          # [B, T]

    bx = b[0].copy()                          # [3U]
    bx[:2 * UN] += b[1][:2 * UN]
    bxT = np.ascontiguousarray(bx.reshape(12, 128).T).astype(np.float32)
    b1h = b[1][2 * UN:]
    b1hbc = np.ascontiguousarray(
        np.broadcast_to(b1h.reshape(4, 128).T[:, :, None], (128, 4, 128))
    ).reshape(128, 512).astype(BF16)

    Wb = W.astype(BF16)
    Ub = U.astype(BF16)
    xb = inputs.astype(BF16)                  # [B, T, D]

    # mask layout per core: mk[g, p(row index in natural), j]
    tgrid = ((np.arange(NGRP)[:, None, None] * CPG +
              np.arange(CPG)[None, :, None]) * CH - HALO +
             np.arange(S)[None, None, :])     # [NGRP, CPG, S]
    valid = (tgrid >= 0)
    tclip = np.clip(tgrid, 0, T - 1)

    in_maps = []
    for ci in range(NCORES):
        bs = slice(ci * BC, (ci + 1) * BC)
        m_core = mask_f[bs]                   # [32, T]
        mk = m_core[:, tclip] * valid[None]   # [32, NGRP, CPG, S]
        mk = np.ascontiguousarray(
            mk.transpose(1, 2, 0, 3).reshape(NGRP, 128, S))
        # mask_bc[g, j, p, (m, r)] = mk[g, r, j]
        mbc = np.ascontiguousarray(
            np.broadcast_to(
                mk.transpose(0, 2, 1)[:, :, None, None, :],
                (NGRP, S, 128, 4, 128))
        ).reshape(NGRP, S, 128, 512).astype(BF16)

        h0T = np.zeros((128, 4, 128), dtype=BF16)
        st = state[bs].astype(BF16)           # [32, 512]
        h0T[:, :, 0:32] = st.T.reshape(4, 128, 32).transpose(1, 0, 2)
        h0T = h0T.reshape(128, 512)

        xs = xb[bs].reshape(BC, NCHUNK, CH, D)      # [b, c, tl, d]
        xTc = np.zeros((NGRP, CH, 128, 512), dtype=BF16)
        for g in range(NGRP):
            arr = xs[:, g * CPG:(g + 1) * CPG]      # [b, cl, tl, d]
            a2 = arr.transpose(2, 3, 1, 0)          # [tl, d, cl, b]
            a3 = a2.reshape(CH, 4, 128, CPG, 32)    # [tl, k, dk, cl, b]
            xTc[g] = a3.transpose(0, 2, 1, 3, 4).reshape(CH, 128, 512)
        in_maps.append({
            "xT": xTc,
            "Wm": Wb, "Um": Ub,
            "bxT": bxT, "b1hbc": b1hbc,
            "mask_bc": mbc, "h0T": h0T,
        })
    return in_maps, mask_f, step_f


def kernel(inputs, dones, state, step, W, U, b):
    from concourse.bass_utils import run_bass_kernel_spmd

    if "nc" not in _nc_cache:
        _nc_cache["nc"] = _build_nc()
    nc = _nc_cache["nc"]

    in_maps, mask_f, step_f = _make_in_maps(inputs, dones, state, step, W, U, b)
    res = run_bass_kernel_spmd(nc, in_maps, core_ids=list(range(NCORES)))
    # outT[g, tl, m, p, r] -> seq[b, t, u]
    outs = []
    for ci in range(NCORES):
        o = res.results[ci]["outT"].astype(np.float32)
        o = o.reshape(NGRP, CH, 4, 128, CPG, 32)
        o = o.transpose(5, 0, 4, 1, 2, 3).reshape(BC, T, UN)
        outs.append(o)
    seq = np.concatenate(outs, axis=0)
    state_f = seq[:, -1, :] * mask_f[:, -1:]
    return seq, state_f, step_f


# revision 28
# speedup vs baseline: 1.2178x; 1.2178x over previous
"""AtariGRU Trainium2 kernel: B=256,T=128,D=U=512, Keras GRU (reset_after) with
done/step-counter resets. Data-parallel over batch on 8 cores; time recurrence
parallelized over T-chunks (len 16 + halo 16) exploiting the guaranteed reset
every <=16 steps.

Fully transposed-state formulation: the hidden state lives as h^T
[feature, row] so the recurrent matmul (stationary = U chunks, moving =
h^T) needs no per-step transposes; gate math runs in the same layout and
emits the next h^T directly. Halo reads become free-dim offsets. bf16
matmuls, f32 PSUM.

kernel(**inputs) takes FULL inputs, returns (sequences, state_f, step_f).
"""
import sys
import numpy as np

for _p in ("/opt/trn_rl_repo", "/root/.axon_site/_ro/trn_rl_repo"):
    if _p not in sys.path:
        sys.path.insert(0, _p)

import ml_dtypes

BF16 = ml_dtypes.bfloat16

# problem constants (hardcoded per harness rules)
B, T, D, UN = 256, 128, 512, 512
NCORES = 8
BC = B // NCORES          # 32 batch rows per core
CH = 16                   # chunk length
HALO = 16                 # halo length (max reset gap)
S = CH + HALO             # steps per chunk
NCHUNK = T // CH          # chunks
NGRP = NCHUNK // 4        # chunk-groups of 4 chunks x 32 rows = 128 rows
CPG = NCHUNK // NGRP      # 4 chunks per group
G3 = 3 * UN               # 1536
MEMORY_SIZE = 16

_nc_cache = {}


def _build_nc():
    import concourse.bacc as bacc
    import concourse.mybir as mybir
    import concourse.tile as tile
    from contextlib import ExitStack

    dt = mybir.dt
    AF = mybir.ActivationFunctionType
    f32 = dt.float32
    bf = dt.bfloat16

    nc = bacc.Bacc(None, target_bir_lowering=False)

    # xT[g, tl, p=d%128, (k=d//128, r)] = x[b(r), t(g,cl(r),tl), d]
    xT_d = nc.declare_dram_parameter("xT", [NGRP, CH, 128, 512], bf, isOutput=False)
    W_d = nc.declare_dram_parameter("Wm", [D, G3], bf, isOutput=False)
    U_d = nc.declare_dram_parameter("Um", [UN, G3], bf, isOutput=False)
    bxT_d = nc.declare_dram_parameter("bxT", [128, 12], f32, isOutput=False)
    b1h_d = nc.declare_dram_parameter("b1hbc", [128, 512], bf, isOutput=False)
    mb_d = nc.declare_dram_parameter("mask_bc", [NGRP, S, 128, 512], bf,
                                     isOutput=False)
    h0_d = nc.declare_dram_parameter("h0T", [128, 512], bf, isOutput=False)
    # outT[g, tl, m, p, r] = h_new[row r, u = m*128+p] at t(g, cl(r), tl)
    out_d = nc.declare_dram_parameter("outT", [NGRP, CH, 4, 128, 128], bf,
                                      isOutput=True)

    with ExitStack() as ctx:
        tc = ctx.enter_context(tile.TileContext(nc))
        const = ctx.enter_context(tc.tile_pool(name="const", bufs=1))
        xmp = ctx.enter_context(tc.tile_pool(name="xmp", bufs=1))
        work = ctx.enter_context(tc.tile_pool(name="work", bufs=2))
        psx = ctx.enter_context(tc.tile_pool(name="psx", bufs=2, space="PSUM"))
        psr = ctx.enter_context(tc.tile_pool(name="psr", bufs=1, space="PSUM"))

        # ---- constants ----
        U_sb = []
        W_sb = []
        for k in range(4):
            ut = const.tile([128, G3], bf, tag=f"U{k}")
            nc.sync.dma_start(out=ut[:], in_=U_d[k * 128:(k + 1) * 128, :])
            U_sb.append(ut)
            wt = const.tile([128, G3], bf, tag=f"W{k}")
            nc.sync.dma_start(out=wt[:], in_=W_d[k * 128:(k + 1) * 128, :])
            W_sb.append(wt)
        bxT = const.tile([128, 12], f32, tag="bxT")
        nc.sync.dma_start(out=bxT[:], in_=bxT_d[:, :])
        b1hbc = const.tile([128, 512], bf, tag="b1hbc")
        nc.sync.dma_start(out=b1hbc[:], in_=b1h_d[:, :])
        h0T = const.tile([128, 512], bf, tag="h0T")
        nc.sync.dma_start(out=h0T[:], in_=h0_d[:, :])

        ident = const.tile([128, 128], bf, tag="ident")
        nc.gpsimd.memset(ident[:], 0.0)
        nc.gpsimd.affine_select(
            out=ident[:], in_=ident[:],
            compare_op=mybir.AluOpType.not_equal,
            fill=1.0, base=0, pattern=[[-1, 128]], channel_multiplier=1,
        )

        # ---- phase 1: xmT[g][p, (m, tl, r)] = (X@W + bias_x)^T ----
        xm_sb = []
        for g in range(NGRP):
            xm = xmp.tile([128, 12 * CH * 128], bf, tag=f"xm{g}")
            xm_sb.append(xm)

        def xm_sl(g, m, tl, r0, r1):
            base = (m * CH + tl) * 128
            return xm_sb[g][:, base + r0:base + r1]

        NTLB = CH // 4
        for g in range(NGRP):
            for tlb in range(NTLB):
                xt4 = work.tile([128, 2048], bf, tag="xt4", bufs=3)
                # dram [tl, p, (k r)] -> sbuf free (k, tl, r)
                nc.sync.dma_start(
                    out=xt4[:],
                    in_=xT_d[g, tlb * 4:(tlb + 1) * 4, :, :].rearrange(
                        "tl p (k r) -> p k tl r", k=4),
                )
                for m in range(12):
                    ps = psx.tile([128, 512], f32, tag="psx")
                    for k in range(4):
                        nc.tensor.matmul(
                            ps[:],
                            lhsT=W_sb[k][:, m * 128:(m + 1) * 128],
                            rhs=xt4[:, k * 512:(k + 1) * 512],
                            start=(k == 0), stop=(k == 3),
                        )
                    # evict with per-partition bias (bias_x folded in free)
                    for tt in range(4):
                        tl = tlb * 4 + tt
                        nc.scalar.activation(
                            xm_sl(g, m, tl, 0, 128),
                            ps[:, tt * 128:(tt + 1) * 128],
                            AF.Identity, bias=bxT[:, m:m + 1],
                        )

        # ---- phase 2: recurrence ----
        carry = []
        for g in range(NGRP):
            c = work.tile([128, UN], bf, tag=f"carry{g}")
            nc.vector.memset(c[:], 0.0)
            carry.append(c)
        mbs = {}
        for g in range(NGRP):
            mb = work.tile([128, UN], bf, tag="mb", bufs=4, name=f"mb_0_{g}")
            nc.sync.dma_start(out=mb[:], in_=mb_d[g, 0, :, :])
            mbs[g] = mb

        for j in range(S):
            main = j >= HALO
            if main:
                tl = j - HALO
                sh = 0
            else:
                nb = (HALO - 1 - j) // CH + 1
                tl = j - HALO + nb * CH
                sh = 32 * nb          # halo row shift

            # --- matmuls for all groups ---
            Ps = []
            for g in range(NGRP):
                P = psr.tile([128, G3], f32, tag=f"P{g}")
                gp = g - 1 if g > 0 else g
                # one m-region at a time: PSUM start-group zeroing is
                # bank-granular, so never interleave open groups in a bank
                for m in range(12):
                    for k in range(4):
                        nc.tensor.matmul(
                            P[:, m * 128:(m + 1) * 128],
                            lhsT=U_sb[k][:, m * 128:(m + 1) * 128],
                            rhs=carry[g][:, k * 128:(k + 1) * 128],
                            start=(k == 0), stop=False,
                        )
                    if m < 8:
                        # fold xm for z,r (rows shifted by sh during halo;
                        # g==0's wrapped rows read finite garbage, masked)
                        nc.tensor.matmul(
                            P[:, m * 128 + sh:(m + 1) * 128],
                            lhsT=ident[:],
                            rhs=xm_sl(g, m, tl, 0, 128 - sh),
                            start=False, stop=(sh == 0),
                        )
                        if sh:
                            nc.tensor.matmul(
                                P[:, m * 128:m * 128 + sh],
                                lhsT=ident[:],
                                rhs=xm_sl(gp, m, tl, 128 - sh, 128),
                                start=False, stop=True,
                            )
                    else:
                        # b1h fold for the h part
                        nc.tensor.matmul(
                            P[:, m * 128:(m + 1) * 128],
                            lhsT=ident[:],
                            rhs=b1hbc[:, (m - 8) * 128:(m - 7) * 128],
                            start=False, stop=True,
                        )
                Ps.append(P)

            # --- gate chain, position-interleaved across groups ---
            rr_, hcs, zgs, dds, zds, hns = {}, {}, {}, {}, {}, {}
            for g in range(NGRP):
                rr = work.tile([128, UN], bf, tag="rg", bufs=4, name=f"rg_{j}_{g}")
                nc.scalar.activation(rr[:], Ps[g][:, 512:1024], AF.Sigmoid)
                rr_[g] = rr
            for g in range(NGRP):
                rh = work.tile([128, UN], bf, tag="rh", bufs=4, name=f"rh_{j}_{g}")
                nc.vector.tensor_mul(rh[:], rr_[g][:], Ps[g][:, 1024:1536])
                rr_[g] = rh
            for g in range(NGRP):
                hcp = work.tile([128, UN], bf, tag="hcpre", bufs=4,
                                name=f"hcp_{j}_{g}")
                gp = g - 1 if g > 0 else g

                def xmh3(gg, r0, r1):
                    return xm_sb[gg][:].rearrange(
                        "p (m tl2 r) -> p m tl2 r", m=12, tl2=CH
                    )[:, 8:12, tl, r0:r1]

                hcp3 = hcp[:].rearrange("p (m r) -> p m r", m=4)
                rh3 = rr_[g][:].rearrange("p (m r) -> p m r", m=4)
                if not main:
                    nc.vector.tensor_add(
                        hcp3[:, :, sh:128], rh3[:, :, sh:128],
                        xmh3(g, 0, 128 - sh),
                    )
                    nc.vector.tensor_add(
                        hcp3[:, :, 0:sh], rh3[:, :, 0:sh],
                        xmh3(gp, 128 - sh, 128),
                    )
                else:
                    nc.vector.tensor_add(hcp3[:], rh3[:], xmh3(g, 0, 128))
                rr_[g] = hcp
            for g in range(NGRP):
                hcs[g] = work.tile([128, UN], bf, tag="hc", bufs=4,
                                   name=f"hc_{j}_{g}")
                nc.scalar.activation(hcs[g][:], rr_[g][:], AF.Tanh)
            for g in range(NGRP):
                zz = work.tile([128, UN], bf, tag="zg", bufs=4, name=f"zg_{j}_{g}")
                nc.scalar.activation(zz[:], Ps[g][:, 0:512], AF.Sigmoid)
                zgs[g] = zz
            for g in range(NGRP):
                dds[g] = work.tile([128, UN], bf, tag="dd", bufs=4,
                                   name=f"dd_{j}_{g}")
                nc.vector.tensor_sub(dds[g][:], carry[g][:], hcs[g][:])
            for g in range(NGRP):
                zds[g] = work.tile([128, UN], bf, tag="zd", bufs=4,
                                   name=f"zd_{j}_{g}")
                nc.vector.tensor_mul(zds[g][:], zgs[g][:], dds[g][:])
            for g in range(NGRP):
                hns[g] = work.tile([128, UN], bf, tag="hnew", bufs=5,
                                   name=f"hn_{j}_{g}")
                nc.vector.tensor_add(hns[g][:], hcs[g][:], zds[g][:])
            for g in range(NGRP):
                cn = work.tile([128, UN], bf, tag=f"carry{g}")
                nc.vector.tensor_mul(cn[:], hns[g][:], mbs[g][:])
                if j == HALO - 1 and g == 0:
                    c2 = work.tile([128, UN], bf, tag="cinj")
                    nc.vector.tensor_add(c2[:], cn[:], h0T[:])
                    cn = c2
                carry[g] = cn
            # prefetch next step's mask tiles; emit outputs
            if j < S - 1:
                for g in range(NGRP):
                    mb = work.tile([128, UN], bf, tag="mb", bufs=4,
                                   name=f"mb_{j + 1}_{g}")
                    nc.sync.dma_start(out=mb[:], in_=mb_d[g, j + 1, :, :])
                    mbs[g] = mb
            if main:
                for g in range(NGRP):
                    nc.gpsimd.dma_start(
                        out=out_d[g, tl, :, :, :].rearrange("m p r -> p m r"),
                        in_=hns[g][:],
                    )

    nc.compile()
    return nc


def _host_scan(dones, step0):
    """reset mask r[B,T] (1=reset after step t) and final step counter."""
    Bn, Tn = dones.shape
    s = step0[:, 0].astype(np.int64).copy()
    r = np.zeros((Bn, Tn), dtype=bool)
    for t in range(Tn):
        s_new = s + 1
        reset = (dones[:, t] == 1) | (s_new % MEMORY_SIZE == 0)
        r[:, t] = reset
        s = np.where(reset, 0, s_new)
    return r, s.astype(np.int32)[:, None]


def _make_in_maps(inputs, dones, state, step, W, U, b):
    r, step_f = _host_scan(dones, step)
    mask_f = (~r).astype(np.float32)          # [B, T]

    bx = b[0].copy()                          # [3U]
    bx[:2 * UN] += b[1][:2 * UN]
    bxT = np.ascontiguousarray(bx.reshape(12, 128).T).astype(np.float32)
    b1h = b[1][2 * UN:]
    b1hbc = np.ascontiguousarray(
        np.broadcast_to(b1h.reshape(4, 128).T[:, :, None], (128, 4, 128))
    ).reshape(128, 512).astype(BF16)

    Wb = W.astype(BF16)
    Ub = U.astype(BF16)
    xb = inputs.astype(BF16)                  # [B, T, D]

    # mask layout per core: mk[g, p(row index in natural), j]
    tgrid = ((np.arange(NGRP)[:, None, None] * CPG +
              np.arange(CPG)[None, :, None]) * CH - HALO +
             np.arange(S)[None, None, :])     # [NGRP, CPG, S]
    valid = (tgrid >= 0)
    tclip = np.clip(tgrid, 0, T - 1)

    in_maps = []
    for ci in range(NCORES):
        bs = slice(ci * BC, (ci + 1) * BC)
        m_core = mask_f[bs]                   # [32, T]
        mk = m_core[:, tclip] * valid[None]   # [32, NGRP, CPG, S]
        mk = np.ascontiguousarray(
            mk.transpose(1, 2, 0, 3).reshape(NGRP, 128, S))
        # mask_bc[g, j, p, (m, r)] = mk[g, r, j]
        mbc = np.ascontiguousarray(
            np.broadcast_to(
                mk.transpose(0, 2, 1)[:, :, None, None, :],
                (NGRP, S, 128, 4, 128))
        ).reshape(NGRP, S, 128, 512).astype(BF16)

        h0T = np.zeros((128, 4, 128), dtype=BF16)
        st = state[bs].astype(BF16)           # [32, 512]
        h0T[:, :, 0:32] = st.T.reshape(4, 128, 32).transpose(1, 0, 2)
        h0T = h0T.reshape(128, 512)

        xs = xb[bs].reshape(BC, NCHUNK, CH, D)      # [b, c, tl, d]
        xTc = np.zeros((NGRP, CH, 128, 512), dtype=BF16)
        for g in range(NGRP):
            arr = xs[:, g * CPG:(g + 1) * CPG]      # [b, cl, tl, d]
            a2 = arr.transpose(2, 3, 1, 0)          # [tl, d, cl, b]
            a3 = a2.reshape(CH, 4, 128, CPG, 32)    # [tl, k, dk, cl, b]
            xTc[g] = a3.transpose(0, 2, 1, 3, 4).reshape(CH, 128, 512)
        in_maps.append({
            "xT": xTc,
            "Wm": Wb, "Um": Ub,
            "bxT": bxT, "b1hbc": b1hbc,
            "mask_bc": mbc, "h0T": h0T,
        })
    return in_maps, mask_f, step_f


def kernel(inputs, dones, state, step, W, U, b):
    from concourse.bass_utils import run_bass_kernel_spmd

    if "nc" not in _nc_cache:
        _nc_cache["nc"] = _build_nc()
    nc = _nc_cache["nc"]

    in_maps, mask_f, step_f = _make_in_maps(inputs, dones, state, step, W, U, b)
    res = run_bass_kernel_spmd(nc, in_maps, core_ids=list(range(NCORES)))
    # outT[g, tl, m, p, r] -> seq[b, t, u]
    outs = []
    for ci in range(NCORES):
        o = res.results[ci]["outT"].astype(np.float32)
        o = o.reshape(NGRP, CH, 4, 128, CPG, 32)
        o = o.transpose(5, 0, 4, 1, 2, 3).reshape(BC, T, UN)
        outs.append(o)
    seq = np.concatenate(outs, axis=0)
    state_f = seq[:, -1, :] * mask_f[:, -1:]
    return seq, state_f, step_f
